# revision 1
# baseline (speedup 1.0000x reference)
"""Trainium2 Bass kernel for GQA attention (RoPE + causal) with output projection.

Strategy: tensor-parallel over heads across 8 NeuronCores. Core c computes
q-heads {2c, 2c+1} and kv-head c//2, projects with its weight slices, runs
causal flash-style attention in scores-transposed layout, applies its slice
of wo, and returns a full-shape partial output. The host sums the 8 partials
(the all-reduce of the TP layout).

All matmuls run as float32r (fp32 data truncated to fp22 in the PE array,
1 cycle/row at free-dim >= 256), accumulating in fp32 PSUM.
"""

import math
from contextlib import ExitStack
from dataclasses import dataclass

import numpy as np

import concourse.bass as bass
import concourse.tile as tile
from concourse import bacc, mybir
from concourse.bass_utils import run_bass_kernel_spmd

F32 = mybir.dt.float32
F32R = mybir.dt.float32r
AF = mybir.ActivationFunctionType
MUL = mybir.AluOpType.mult
ADD = mybir.AluOpType.add


@dataclass(frozen=True)
class Cfg:
    B: int = 4          # batch
    S: int = 2048       # sequence length
    D: int = 2048       # model dim
    HQC: int = 2        # q-heads per core
    HD: int = 128       # head dim (must be 128)
    QCH: int = 512      # q-chunk (matmul moving free dim)

    @property
    def DT(self):
        return self.D // 128   # d-tiles

    @property
    def KT(self):
        return self.S // 128   # k-tiles / s-tiles / q-tiles

    @property
    def NQC(self):
        return self.S // self.QCH  # q-chunks

    @property
    def RB(self):
        return self.QCH // 128     # band tiles per q-chunk

    @property
    def NDC(self):
        return self.D // self.QCH  # dout chunks


def r(ap):
    """View an fp32 AP as float32r for full-rate PE matmuls."""
    return ap.bitcast(F32R)


def build_program(cfg: Cfg):
    """Build + compile the single-core Bass program (same program on every core)."""
    c = cfg
    assert c.HD == 128
    nc = bacc.Bacc("TRN2", target_bir_lowering=False, debug=False)

    xt_d = nc.dram_tensor("xt", [c.B, c.D, c.S], F32, kind="ExternalInput")
    wqt_d = nc.dram_tensor("wqt", [c.D, c.HQC * c.HD], F32, kind="ExternalInput")
    wkt_d = nc.dram_tensor("wkt", [c.D, c.HD], F32, kind="ExternalInput")
    wvt_d = nc.dram_tensor("wvt", [c.D, c.HD], F32, kind="ExternalInput")
    wot_d = nc.dram_tensor("wot", [c.HQC * c.HD, c.D], F32, kind="ExternalInput")
    ra_d = nc.dram_tensor("ra", [c.HD, c.S], F32, kind="ExternalInput")
    rb_d = nc.dram_tensor("rb", [c.HD, c.S], F32, kind="ExternalInput")
    cm_d = nc.dram_tensor("cm", [c.RB, 128, c.QCH], F32, kind="ExternalInput")
    id_d = nc.dram_tensor("id", [128, 128], F32, kind="ExternalInput")
    pm_d = nc.dram_tensor("pm", [128, 128], F32, kind="ExternalInput")
    onec_d = nc.dram_tensor("onec", [128, 1], F32, kind="ExternalInput")
    oner_d = nc.dram_tensor("oner", [1, 128], F32, kind="ExternalInput")
    out_d = nc.dram_tensor("partial", [c.B, c.S, c.D], F32, kind="ExternalOutput")

    scale = 1.0 / math.sqrt(c.HD)

    with tile.TileContext(nc) as tc, ExitStack() as ctx:
        const = ctx.enter_context(tc.tile_pool(name="const", bufs=1))
        xpool = ctx.enter_context(tc.tile_pool(name="xp", bufs=4))
        qkv = ctx.enter_context(tc.tile_pool(name="qkv", bufs=1))
        ptp = ctx.enter_context(tc.tile_pool(name="ptp", bufs=8))
        rp = ctx.enter_context(tc.tile_pool(name="rp", bufs=2))
        zp = ctx.enter_context(tc.tile_pool(name="zp", bufs=2))
        atp = ctx.enter_context(tc.tile_pool(name="atp", bufs=1))
        orp = ctx.enter_context(tc.tile_pool(name="orp", bufs=2))
        ps = ctx.enter_context(
            tc.tile_pool(name="ps", bufs=4, space=bass.MemorySpace.PSUM)
        )
        pj = ctx.enter_context(
            tc.tile_pool(name="pj", bufs=4, space=bass.MemorySpace.PSUM)
        )

        # ---- resident constants ----
        wq_sb = const.tile([128, c.DT, c.HQC * c.HD], F32R, name="wq_sb")
        nc.sync.dma_start(wq_sb[:], r(wqt_d.rearrange("(t p) h -> p t h", p=128)))
        wk_sb = const.tile([128, c.DT, c.HD], F32R, name="wk_sb")
        nc.sync.dma_start(wk_sb[:], r(wkt_d.rearrange("(t p) h -> p t h", p=128)))
        wv_sb = const.tile([128, c.DT, c.HD], F32R, name="wv_sb")
        nc.sync.dma_start(wv_sb[:], r(wvt_d.rearrange("(t p) h -> p t h", p=128)))
        wo_sb = const.tile([128, c.HQC, c.D], F32R, name="wo_sb")
        nc.sync.dma_start(wo_sb[:], r(wot_d.rearrange("(h p) d -> p h d", p=128)))
        ra_sb = const.tile([128, c.S], F32, name="ra_sb")
        nc.sync.dma_start(ra_sb[:], ra_d[:])
        rb_sb = const.tile([128, c.S], F32, name="rb_sb")
        nc.sync.dma_start(rb_sb[:], rb_d[:])
        cm_sb = const.tile([128, c.RB, c.QCH], F32, name="cm_sb")
        nc.sync.dma_start(cm_sb[:], cm_d.rearrange("m p q -> p m q"))
        id_sb = const.tile([128, 128], F32, name="id_sb")
        nc.sync.dma_start(id_sb[:], id_d[:])
        pm_sb = const.tile([128, 128], F32R, name="pm_sb")
        nc.sync.dma_start(pm_sb[:], r(pm_d[:]))
        ones_c = const.tile([128, 1], F32R, name="ones_c")
        nc.sync.dma_start(ones_c[:], r(onec_d[:]))
        ones_r = const.tile([1, 128], F32R, name="ones_r")
        nc.sync.dma_start(ones_r[:], r(oner_d[:]))

        def rope(t):
            # t[p] = t[p]*ra[p] + t[partner(p)]*rb[p]; the cross-partition
            # partner swap runs on the PE via a pair-swap permutation matmul
            # (DVE lanes are partition-locked, so it can't shift partitions).
            for ch in range(c.NQC):
                sl = slice(ch * c.QCH, (ch + 1) * c.QCH)
                rps = ps.tile([128, c.QCH], F32, name="rps", tag="ps")
                nc.tensor.matmul(rps[:], r(pm_sb[:]), r(t[:, sl]))
                swp = ptp.tile([128, c.QCH], F32, name="swp", tag="pt")
                nc.vector.tensor_tensor(swp[:], rps[:], rb_sb[:, sl], MUL)
                nc.vector.tensor_tensor(t[:, sl], t[:, sl], ra_sb[:, sl], MUL)
                nc.vector.tensor_tensor(t[:, sl], t[:, sl], swp[:], ADD)

        for b in range(c.B):
            # ============ Phase 1: Q^T / K^T / V^T projections ============
            qts = [
                qkv.tile([128, c.S], F32R, name=f"qt{h}", tag=f"qt{h}", bufs=2)
                for h in range(c.HQC)
            ]
            kt_sb = qkv.tile([128, c.S], F32R, name="kt_sb", tag="kt_sb", bufs=2)
            vt_sb = qkv.tile([128, c.S], F32, name="vt_sb", tag="vt_sb")

            for sc in range(c.NQC):
                sl = slice(sc * c.QCH, (sc + 1) * c.QCH)
                acc = [
                    pj.tile([128, c.QCH], F32, name=f"pj{i}", tag="pj")
                    for i in range(c.HQC + 2)
                ]
                for dt in range(c.DT):
                    xt = xpool.tile([128, c.QCH], F32R, name="xt_t", tag="xt_t")
                    nc.sync.dma_start(xt[:], r(xt_d[b, dt * 128:(dt + 1) * 128, sl]))
                    st, sp = dt == 0, dt == c.DT - 1
                    for h in range(c.HQC):
                        nc.tensor.matmul(
                            acc[h][:],
                            r(wq_sb[:, dt, h * c.HD:(h + 1) * c.HD]),
                            r(xt[:]), start=st, stop=sp,
                        )
                    nc.tensor.matmul(
                        acc[c.HQC][:], r(wk_sb[:, dt, :]), r(xt[:]), start=st, stop=sp
                    )
                    nc.tensor.matmul(
                        acc[c.HQC + 1][:], r(wv_sb[:, dt, :]), r(xt[:]), start=st, stop=sp
                    )
                for h in range(c.HQC):
                    nc.scalar.copy(qts[h][:, sl], acc[h][:])
                nc.scalar.copy(kt_sb[:, sl], acc[c.HQC][:])
                nc.scalar.copy(vt_sb[:, sl], acc[c.HQC + 1][:])

            # rope on Q heads and K
            for t in qts + [kt_sb]:
                rope(t)

            # V^T -> V natural via PE transposes
            vn = qkv.tile([128, c.KT, c.HD], F32R, name="vn", tag="vn")
            for st_i in range(c.KT):
                tp = ps.tile([128, 128], F32, name="tp", tag="ps")
                nc.tensor.transpose(
                    tp[:], vt_sb[:, st_i * 128:(st_i + 1) * 128], id_sb[:]
                )
                nc.scalar.copy(vn[:, st_i, :], tp[:])

            # ============ Phase 2: causal attention, S^T layout ============
            ats = [
                atp.tile([128, c.S], F32R, name=f"at{h}", tag=f"at{h}")
                for h in range(c.HQC)
            ]
            for h in range(c.HQC):
                qt = qts[h]
                for qc in range(c.NQC):
                    qsl = slice(qc * c.QCH, (qc + 1) * c.QCH)
                    nkt = c.RB * (qc + 1)
                    ot = ps.tile([128, c.QCH], F32, name="ot", tag="ps")
                    rsum = rp.tile([128, c.QCH], F32R, name="rsum", tag="rsum")
                    for kt in range(nkt):
                        stp = ps.tile([128, c.QCH], F32, name="stp", tag="ps")
                        nc.tensor.matmul(
                            stp[:],
                            r(kt_sb[:, kt * 128:(kt + 1) * 128]),
                            r(qt[:, qsl]),
                        )
                        pt = ptp.tile([128, c.QCH], F32R, name="pt", tag="pt")
                        nc.scalar.activation(pt[:], stp[:], AF.Exp, scale=scale)
                        ridx = kt - (nkt - c.RB)
                        if ridx >= 0:  # diagonal band: causal mask.
                            # columns >= 128*(ridx+1) are all-ones -> skip them
                            w = 128 * (ridx + 1)
                            nc.vector.tensor_tensor(
                                pt[:, 0:w], pt[:, 0:w], cm_sb[:, ridx, 0:w], MUL
                            )
                        if kt == 0:
                            nc.vector.tensor_copy(rsum[:], pt[:])
                        else:
                            nc.vector.tensor_tensor(rsum[:], rsum[:], pt[:], ADD)
                        nc.tensor.matmul(
                            ot[:], r(vn[:, kt, :]), r(pt[:]),
                            start=(kt == 0), stop=(kt == nkt - 1),
                        )
                    # softmax denominator: column-sum of rsum, reciprocal,
                    # broadcast back to 128 partitions via K=1 matmul
                    zps = ps.tile([1, c.QCH], F32, name="zps", tag="ps")
                    nc.tensor.matmul(zps[:], r(ones_c[:]), r(rsum[:]))
                    zr = zp.tile([1, c.QCH], F32R, name="zr", tag="zr")
                    with nc.allow_low_precision("fp22 softmax denominator"):
                        nc.vector.reciprocal(zr[:], zps[:])
                    zbp = ps.tile([128, c.QCH], F32, name="zbp", tag="ps")
                    nc.tensor.matmul(zbp[:], r(ones_r[:]), r(zr[:]))
                    zb = zp.tile([128, c.QCH], F32, name="zb", tag="zb")
                    nc.scalar.copy(zb[:], zbp[:])
                    nc.vector.tensor_tensor(ats[h][:, qsl], ot[:], zb[:], MUL)

            # ============ Phase 3: output projection (partial of wo) ============
            for qt_i in range(c.KT):
                orow = orp.tile([128, c.D], F32, name="orow", tag="orow")
                for dc in range(c.NDC):
                    dsl = slice(dc * c.QCH, (dc + 1) * c.QCH)
                    o3 = ps.tile([128, c.QCH], F32, name="o3", tag="ps")
                    for h in range(c.HQC):
                        nc.tensor.matmul(
                            o3[:],
                            r(ats[h][:, qt_i * 128:(qt_i + 1) * 128]),
                            r(wo_sb[:, h, dsl]),
                            start=(h == 0), stop=(h == c.HQC - 1),
                        )
                    if dc % 2 == 0:
                        nc.scalar.copy(orow[:, dsl], o3[:])
                    else:
                        nc.vector.tensor_copy(orow[:, dsl], o3[:])
                nc.sync.dma_start(
                    out_d[b, qt_i * 128:(qt_i + 1) * 128, :], orow[:]
                )

    nc.compile()
    nc.finalize()
    return nc


# ---------------------------------------------------------------------------
# Host-side sharding / gathering
# ---------------------------------------------------------------------------

def host_prep(x, freq_cis, wq, wk, wv, wo, n_cores, cfg: Cfg):
    """Build per-core input maps (numpy only)."""
    c = cfg
    B, S, D, HD, HQC = c.B, c.S, c.D, c.HD, c.HQC
    H = wq.shape[0] // HD
    HKV = wk.shape[0] // HD
    rep = H // HKV

    x = np.asarray(x, np.float32)
    freq_cis = np.asarray(freq_cis, np.float32)
    wq = np.asarray(wq, np.float32)
    wk = np.asarray(wk, np.float32)
    wv = np.asarray(wv, np.float32)
    wo = np.asarray(wo, np.float32)

    xT = np.ascontiguousarray(x.transpose(0, 2, 1))  # [B, D, S]

    # rope tables, interleaved layout: out[p] = ra[p]*t[p] + rb[p]*t[partner(p)]
    # with partner(2p) = 2p+1, partner(2p+1) = 2p
    a = freq_cis[:, :, 0, 0].T  # [HD/2, S]
    bb = freq_cis[:, :, 0, 1].T
    cc = freq_cis[:, :, 1, 0].T
    dd = freq_cis[:, :, 1, 1].T
    S_ = freq_cis.shape[0]
    ra = np.empty((HD, S_), np.float32)
    rb = np.empty((HD, S_), np.float32)
    ra[0::2], ra[1::2] = a, dd
    rb[0::2], rb[1::2] = bb, cc

    # pair-swap permutation matrix (symmetric involution)
    pm = np.zeros((HD, HD), np.float32)
    idx = np.arange(HD)
    pm[idx, idx ^ 1] = 1.0

    # causal band masks: cm[m, k, q] = 1 if (k + 128*m) <= q
    ks = np.arange(128)[:, None]
    qs = np.arange(c.QCH)[None, :]
    cm = np.stack(
        [(ks + 128 * m <= qs).astype(np.float32) for m in range(c.RB)], axis=0
    )
    ident = np.eye(128, dtype=np.float32)

    in_maps = []
    for core in range(n_cores):
        h0 = core * HQC
        kvh = h0 // rep
        wq_c = wq[h0 * HD:(h0 + HQC) * HD]
        wk_c = wk[kvh * HD:(kvh + 1) * HD]
        wv_c = wv[kvh * HD:(kvh + 1) * HD]
        wo_c = wo[:, h0 * HD:(h0 + HQC) * HD]
        in_maps.append({
            "xt": xT,
            "wqt": np.ascontiguousarray(wq_c.T),
            "wkt": np.ascontiguousarray(wk_c.T),
            "wvt": np.ascontiguousarray(wv_c.T),
            "wot": np.ascontiguousarray(wo_c.T),
            "ra": ra,
            "rb": rb,
            "cm": cm,
            "id": ident,
            "pm": pm,
            "onec": np.ones((HD, 1), np.float32),
            "oner": np.ones((1, HD), np.float32),
        })
    return in_maps


def run(inputs: dict, n_cores: int = 8, cfg: Cfg = Cfg(), trace: bool = False):
    in_maps = host_prep(
        inputs["x"], inputs["freq_cis"], inputs["wq"], inputs["wk"],
        inputs["wv"], inputs["wo"], n_cores, cfg,
    )
    nc = build_program(cfg)
    res = run_bass_kernel_spmd(nc, in_maps, list(range(n_cores)), trace=trace)
    out = res.results[0]["partial"].astype(np.float64)
    for core in range(1, n_cores):
        out += res.results[core]["partial"]
    return out.astype(np.float32), res


def kernel(**inputs) -> np.ndarray:
    out, _ = run(inputs, n_cores=8, cfg=Cfg())
    return out



# revision 11
# speedup vs baseline: 1.6959x; 1.6959x over previous
"""Trainium2 Bass kernel for GQA attention (RoPE + causal) + output projection.

Sharding: (batch, head-half) across 8 cores. Core c handles batch c//2 and
q-heads [8*(c%2), 8*(c%2)+8) with kv-heads {2*(c%2), 2*(c%2)+1}. Each core
writes a transposed partial output [D, S] in bf16; the host sums core pairs
and transposes back.

Engine plan (per core):
- PE: projections (bf16), rope pair-swap, scores/AV (bf16), softmax-denominator
  column sums + broadcasts, output projection (bf16). Emission is software-
  pipelined (drains overlapped into following matmul streams, AV lookahead,
  deferred softmax tails) to keep the PE stream dense so it holds the 2.4 GHz
  p-state.
- Scalar (Act): some PSUM->SBUF rope copies, exp (bf16 out), vn copies.
- DVE: rope swap-mult, causal mask mult (bf16 2x), softmax partial-sum adds
  (bf16 2x, two interleaved accumulators combined for free on the PE),
  reciprocal_approx_fast, final normalize.
- Pool (gpsimd): rope mult/add, V copies, P3 PSUM->SBUF output copies.
"""

import math
from contextlib import ExitStack
from dataclasses import dataclass

import numpy as np

import concourse.bass as bass
import concourse.tile as tile
from concourse import bacc, mybir
from concourse.bass_utils import run_bass_kernel_spmd

F32 = mybir.dt.float32
F32R = mybir.dt.float32r
BF16 = mybir.dt.bfloat16
AF = mybir.ActivationFunctionType
MUL = mybir.AluOpType.mult
ADD = mybir.AluOpType.add


@dataclass(frozen=True)
class Cfg:
    B: int = 4          # batch
    S: int = 2048       # sequence length
    D: int = 2048       # model dim
    HQC: int = 8        # q-heads per core
    HD: int = 128       # head dim
    QCH: int = 512      # chunk (matmul moving free dim)
    LOOK: int = 2       # AV lookahead in the attention pipeline

    @property
    def DT(self):
        return self.D // 128   # d-tiles

    @property
    def KT(self):
        return self.S // 128   # 128-row tiles along S

    @property
    def NQC(self):
        return self.S // self.QCH  # q-chunks

    @property
    def RB(self):
        return self.QCH // 128     # band tiles per q-chunk


def r(ap):
    """View an fp32 AP as float32r for full-rate PE matmuls."""
    return ap.bitcast(F32R)


def build_program(cfg: Cfg):
    c = cfg
    assert c.HD == 128 and c.HQC == 8
    nc = bacc.Bacc("TRN2", target_bir_lowering=False, debug=False)

    xt_d = nc.dram_tensor("xt", [c.D, c.S], BF16, kind="ExternalInput")
    wq_d = nc.dram_tensor("wq", [2, c.DT, 128, 4 * c.HD], BF16, kind="ExternalInput")
    wk_d = nc.dram_tensor("wk", [c.DT, 128, 2 * c.HD], BF16, kind="ExternalInput")
    wv_d = nc.dram_tensor("wv", [c.DT, 128, 2 * c.HD], BF16, kind="ExternalInput")
    wo_d = nc.dram_tensor("wo", [c.DT, 128, c.HQC * c.HD], BF16, kind="ExternalInput")
    ra_d = nc.dram_tensor("ra", [c.HD, c.S], F32, kind="ExternalInput")
    rb_d = nc.dram_tensor("rb", [c.HD, c.S], F32, kind="ExternalInput")
    cm_d = nc.dram_tensor("cm", [128, c.RB, c.QCH], BF16, kind="ExternalInput")
    pm_d = nc.dram_tensor("pm", [128, 128], F32, kind="ExternalInput")
    idn_d = nc.dram_tensor("idn", [128, 128], BF16, kind="ExternalInput")
    onec_d = nc.dram_tensor("onec", [c.HD, 1], BF16, kind="ExternalInput")
    oner_d = nc.dram_tensor("oner", [1, c.HD], BF16, kind="ExternalInput")
    out_d = nc.dram_tensor("partialT", [c.D, c.S], BF16, kind="ExternalOutput")

    scale = 1.0 / math.sqrt(c.HD)

    with tile.TileContext(nc) as tc, ExitStack() as ctx:
        ctx.enter_context(nc.allow_low_precision("bf16 internals; tol 2e-2"))
        const = ctx.enter_context(tc.tile_pool(name="const", bufs=1))
        wp = ctx.enter_context(tc.tile_pool(name="wp", bufs=1))
        wop = ctx.enter_context(tc.tile_pool(name="wop", bufs=3))
        xp = ctx.enter_context(tc.tile_pool(name="xp", bufs=6))
        qkp = ctx.enter_context(tc.tile_pool(name="qkp", bufs=1))
        rtp = ctx.enter_context(tc.tile_pool(name="rtp", bufs=1))
        ptp = ctx.enter_context(tc.tile_pool(name="ptp", bufs=6))
        rp = ctx.enter_context(tc.tile_pool(name="rp", bufs=2))
        ocp = ctx.enter_context(tc.tile_pool(name="ocp", bufs=3))
        ps = ctx.enter_context(
            tc.tile_pool(name="ps", bufs=1, space=bass.MemorySpace.PSUM)
        )

        def psum(shape, tag, bufs, name):
            return ps.tile(shape, F32, name=name, tag=tag, bufs=bufs)

        # ---- constants ----
        ra_sb = const.tile([128, c.S], F32, name="ra_sb")
        nc.sync.dma_start(ra_sb[:], ra_d[:])
        rb_sb = const.tile([128, c.S], F32, name="rb_sb")
        nc.sync.dma_start(rb_sb[:], rb_d[:])
        cm_sb = const.tile([128, c.RB, c.QCH], BF16, name="cm_sb")
        nc.sync.dma_start(cm_sb[:], cm_d[:])
        pm_sb = const.tile([128, 128], F32R, name="pm_sb")
        nc.sync.dma_start(pm_sb[:], r(pm_d[:]))
        idn_sb = const.tile([128, 128], BF16, name="idn_sb")
        nc.sync.dma_start(idn_sb[:], idn_d[:])
        onec_sb = const.tile([128, 1], BF16, name="onec_sb")
        nc.sync.dma_start(onec_sb[:], onec_d[:])
        oner_sb = const.tile([1, 128], BF16, name="oner_sb")
        nc.sync.dma_start(oner_sb[:], oner_d[:])

        # ---- resident weights / activations ----
        wq_sb = [
            wp.tile([128, c.DT, 4 * c.HD], BF16, name=f"wq{g}", tag=f"wq{g}")
            for g in range(2)
        ]
        wk_sb = wp.tile([128, c.DT, 2 * c.HD], BF16, name="wk_sb")
        wv_sb = wp.tile([128, c.DT, 2 * c.HD], BF16, name="wv_sb")

        q_bf = [
            qkp.tile([128, c.S], BF16, name=f"q{h}", tag=f"q{h}") for h in range(8)
        ]
        k_bf = [
            qkp.tile([128, c.S], BF16, name=f"k{g}", tag=f"k{g}") for g in range(2)
        ]
        vt = [
            qkp.tile([128, c.S], BF16, name=f"vt{g}", tag=f"vt{g}") for g in range(2)
        ]
        vn = [
            qkp.tile([128, c.KT, c.HD], BF16, name=f"vn{g}", tag=f"vn{g}")
            for g in range(2)
        ]
        ats = [
            qkp.tile([128, c.S], BF16, name=f"at{h}", tag=f"at{h}") for h in range(8)
        ]

        # ================= Phase 1: projections (+rope, V transpose) ========
        # Drains for s-chunk sc are emitted at the TOP of the next chunk's
        # loop (before the PSUM accumulators are re-allocated) so the WAR
        # dependencies are visible to the tile framework. The first-read
        # copies are spread across Scalar/DVE/Pool so the PE's stall at the
        # chunk boundary stays short; the pair-swap matmuls then give the PE
        # immediate work while the copies drain.
        def emit_drains(g, sc, sl, accs):
            ts = []
            for i in range(4):  # q heads: scalar/scalar/dve/dve copies
                t = rtp.tile([128, c.QCH], F32R, name=f"t{g}{sc}{i}",
                             tag="rt", bufs=3)
                if i < 2:
                    nc.scalar.copy(t[:], accs[i][:])
                else:
                    nc.vector.tensor_copy(t[:], accs[i][:])
                ts.append(t)
            tk = rtp.tile([128, c.QCH], F32R, name=f"tk{g}{sc}", tag="rt", bufs=3)
            nc.scalar.copy(tk[:], accs[4][:])
            ts.append(tk)
            # Pool cannot read PSUM; V drain goes on DVE (casts to bf16)
            nc.vector.tensor_copy(vt[g][:, sl], accs[5][:])
            # rope the 5 copied tensors (4 Q + K)
            dsts = [q_bf[g * 4 + i] for i in range(4)] + [k_bf[g]]
            for i, (t, dst) in enumerate(zip(ts, dsts)):
                rps = psum([128, c.QCH], "zz", 2, f"rps{g}{sc}{i}")
                nc.tensor.matmul(rps[:], pm_sb[:], t[:])
                sw = rtp.tile([128, c.QCH], F32, name=f"sw{g}{sc}{i}",
                              tag="sw", bufs=3)
                nc.vector.tensor_tensor(sw[:], rps[:], rb_sb[:, sl], MUL)
                tr = rtp.tile([128, c.QCH], F32, name=f"tr{g}{sc}{i}",
                              tag="tr", bufs=2)
                nc.gpsimd.tensor_tensor(tr[:], t[:], ra_sb[:, sl], MUL)
                nc.gpsimd.tensor_tensor(dst[:, sl], tr[:], sw[:], ADD)

        def make_transposes(g):
            out = []
            for st_i in range(c.KT):
                def tr_one(st_i=st_i, g=g):
                    tp = ps.tile([128, 128], BF16, name=f"tp{g}{st_i}",
                                 tag="zz", bufs=2)
                    nc.tensor.transpose(
                        tp[:], vt[g][:, st_i * 128:(st_i + 1) * 128], idn_sb[:]
                    )
                    nc.scalar.copy(vn[g][:, st_i, :], tp[:])
                out.append(tr_one)
            return out

        pending_drain = None   # (g, sc, sl, accs) of the previous chunk
        pending_tr = []        # deferred V-transpose closures

        for g in range(2):
            for sc in range(c.NQC):
                sl = slice(sc * c.QCH, (sc + 1) * c.QCH)
                if pending_drain is not None:
                    emit_drains(*pending_drain)
                    pending_drain = None
                accs = [
                    psum([128, c.QCH], "stp" if i < 4 else "ot", 4 if i < 4 else 2,
                         f"acc{g}{sc}{i}")
                    for i in range(6)
                ]
                for dt in range(c.DT):
                    if g == 0 and sc == 0:
                        nc.sync.dma_start(wq_sb[0][:, dt, :], wq_d[0, dt])
                        nc.sync.dma_start(wk_sb[:, dt, :], wk_d[dt])
                        nc.sync.dma_start(wv_sb[:, dt, :], wv_d[dt])
                    elif g == 0 and sc == 1:
                        nc.sync.dma_start(wq_sb[1][:, dt, :], wq_d[1, dt])
                    xt_t = xp.tile([128, c.QCH], BF16, name="xt_t", tag="xt")
                    nc.sync.dma_start(xt_t[:], xt_d[dt * 128:(dt + 1) * 128, sl])
                    st, sp = dt == 0, dt == c.DT - 1
                    for i in range(4):
                        nc.tensor.matmul(
                            accs[i][:],
                            wq_sb[g][:, dt, i * c.HD:(i + 1) * c.HD],
                            xt_t[:], start=st, stop=sp,
                        )
                    nc.tensor.matmul(
                        accs[4][:], wk_sb[:, dt, g * c.HD:(g + 1) * c.HD],
                        xt_t[:], start=st, stop=sp,
                    )
                    nc.tensor.matmul(
                        accs[5][:], wv_sb[:, dt, g * c.HD:(g + 1) * c.HD],
                        xt_t[:], start=st, stop=sp,
                    )
                    # interleave deferred V transposes of the previous group
                    if pending_tr and dt % 2 == 1:
                        pending_tr.pop(0)()
                pending_drain = (g, sc, sl, accs)
            # chunk sc=3 drains; then queue this group's V transposes
            emit_drains(*pending_drain)
            pending_drain = None
            pending_tr.extend(make_transposes(g))

        # ================= Phase 2: causal attention =======================
        with nc.allow_low_precision("bf16 softmax partial sums"):
            prev_tail = None
            blocks = [(g, h, qc) for g in range(2) for h in range(4)
                      for qc in range(c.NQC)]
            for g, h, qc in blocks:
                qh = g * 4 + h
                qsl = slice(qc * c.QCH, (qc + 1) * c.QCH)
                nkt = c.RB * (qc + 1)
                ot = psum([128, c.QCH], "ot", 2, f"ot{qh}{qc}")
                rs = [
                    rp.tile([128, c.QCH], BF16, name=f"rs{i}_{qh}{qc}",
                            tag=f"rs{i}", bufs=2)
                    for i in range(2)
                ]
                av_q = []
                for kt in range(nkt):
                    stp = psum([128, c.QCH], "stp", 4, f"stp{qh}{qc}{kt}")
                    nc.tensor.matmul(
                        stp[:],
                        k_bf[g][:, kt * 128:(kt + 1) * 128],
                        q_bf[qh][:, qsl],
                    )
                    pt = ptp.tile([128, c.QCH], BF16, name="pt", tag="pt")
                    nc.scalar.activation(pt[:], stp[:], AF.Exp, scale=scale)
                    ridx = kt - (nkt - c.RB)
                    if ridx >= 0:  # diagonal band: causal mask
                        w = 128 * (ridx + 1)
                        nc.vector.tensor_tensor(
                            pt[:, 0:w], pt[:, 0:w], cm_sb[:, ridx, 0:w], MUL
                        )
                    p = kt % 2
                    if kt < 2:
                        nc.vector.tensor_copy(rs[p][:], pt[:])
                    else:
                        nc.vector.tensor_tensor(rs[p][:], rs[p][:], pt[:], ADD)
                    av_q.append((kt, pt))
                    if len(av_q) > c.LOOK:
                        k2, p2 = av_q.pop(0)
                        nc.tensor.matmul(
                            ot[:], vn[g][:, k2, :], p2[:],
                            start=(k2 == 0), stop=(k2 == nkt - 1),
                        )
                    if kt == 1 and prev_tail is not None:
                        prev_tail()
                        prev_tail = None
                    if kt == 2 and pending_tr:
                        pending_tr.pop(0)()
                        if pending_tr:
                            pending_tr.pop(0)()
                for k2, p2 in av_q:
                    nc.tensor.matmul(
                        ot[:], vn[g][:, k2, :], p2[:],
                        start=(k2 == 0), stop=(k2 == nkt - 1),
                    )

                def make_tail(ot=ot, rs=rs, qh=qh, qsl=qsl, qc=qc):
                    def tail():
                        zz = psum([1, c.QCH], "zz", 2, f"zz{qh}{qc}")
                        nc.tensor.matmul(zz[:], onec_sb[:], rs[0][:],
                                         start=True, stop=False)
                        nc.tensor.matmul(zz[:], onec_sb[:], rs[1][:],
                                         start=False, stop=True)
                        zr = rp.tile([1, c.QCH], F32, name=f"zr{qh}{qc}",
                                     tag="zr", bufs=2)
                        nc.vector.reciprocal_approx_fast(zr[:], zz[:])
                        zrb = rp.tile([1, c.QCH], BF16, name=f"zrb{qh}{qc}",
                                      tag="zrb", bufs=2)
                        nc.scalar.copy(zrb[:], zr[:])
                        zbp = psum([128, c.QCH], "zz", 2, f"zbp{qh}{qc}")
                        nc.tensor.matmul(zbp[:], oner_sb[:], zrb[:])
                        # DVE can't take two PSUM operands: stage zb in SBUF
                        zb = rp.tile([128, c.QCH], BF16, name=f"zb{qh}{qc}",
                                     tag="zb", bufs=2)
                        nc.scalar.copy(zb[:], zbp[:])
                        nc.vector.tensor_tensor(
                            ats[qh][:, qsl], ot[:], zb[:], MUL
                        )
                    return tail

                prev_tail = make_tail()
            prev_tail()

        # ================= Phase 3: output projection (transposed) =========
        wo_tiles = []
        for dt in range(3):
            wt = wop.tile([128, c.HQC * c.HD], BF16, name="wo_t", tag="wo")
            nc.sync.dma_start(wt[:], wo_d[dt])
            wo_tiles.append(wt)
        for dt in range(c.DT):
            wt = wo_tiles.pop(0)
            if dt + 3 < c.DT:
                nwt = wop.tile([128, c.HQC * c.HD], BF16, name="wo_t", tag="wo")
                nc.sync.dma_start(nwt[:], wo_d[dt + 3])
                wo_tiles.append(nwt)
            for qc in range(c.NQC):
                qsl = slice(qc * c.QCH, (qc + 1) * c.QCH)
                oT = psum([128, c.QCH], "stp", 4, f"oT{dt}{qc}")
                for h8 in range(c.HQC):
                    nc.tensor.matmul(
                        oT[:],
                        wt[:, h8 * c.HD:(h8 + 1) * c.HD],
                        ats[h8][:, qsl],
                        start=(h8 == 0), stop=(h8 == c.HQC - 1),
                    )
                oc = ocp.tile([128, c.QCH], BF16, name="oc", tag="oc")
                if qc % 2 == 0:
                    nc.scalar.copy(oc[:], oT[:])
                else:
                    nc.vector.tensor_copy(oc[:], oT[:])
                nc.sync.dma_start(
                    out_d[dt * 128:(dt + 1) * 128, qsl], oc[:]
                )

    nc.compile()
    nc.finalize()
    return nc


# ---------------------------------------------------------------------------
# Host-side sharding / gathering
# ---------------------------------------------------------------------------

def host_prep(x, freq_cis, wq, wk, wv, wo, n_cores, cfg: Cfg):
    import ml_dtypes
    BF = ml_dtypes.bfloat16
    c = cfg
    HD, HQC = c.HD, c.HQC

    x = np.asarray(x, np.float32)
    freq_cis = np.asarray(freq_cis, np.float32)
    wq = np.asarray(wq, np.float32)
    wk = np.asarray(wk, np.float32)
    wv = np.asarray(wv, np.float32)
    wo = np.asarray(wo, np.float32)

    # rope tables, interleaved layout: out[p] = ra[p]*t[p] + rb[p]*t[partner(p)]
    a = freq_cis[:, :, 0, 0].T
    bb = freq_cis[:, :, 0, 1].T
    cc = freq_cis[:, :, 1, 0].T
    dd = freq_cis[:, :, 1, 1].T
    S_ = freq_cis.shape[0]
    ra = np.empty((HD, S_), np.float32)
    rb = np.empty((HD, S_), np.float32)
    ra[0::2], ra[1::2] = a, dd
    rb[0::2], rb[1::2] = bb, cc

    pm = np.zeros((HD, HD), np.float32)
    idx = np.arange(HD)
    pm[idx, idx ^ 1] = 1.0

    # causal band masks: cm[k, m, q] = 1 if (k + 128*m) <= q
    ks = np.arange(128)[:, None]
    qs = np.arange(c.QCH)[None, :]
    cm = np.stack(
        [(ks + 128 * m <= qs) for m in range(c.RB)], axis=1
    ).astype(BF)

    in_maps = []
    for core in range(n_cores):
        b, hh = core // 2, core % 2
        hq0 = hh * HQC
        xt = np.ascontiguousarray(x[b].T.astype(BF))

        wq_c = wq[hq0 * HD:(hq0 + HQC) * HD]              # [1024, D]
        wq_p = np.ascontiguousarray(
            wq_c.T.reshape(c.DT, 128, 2, 4 * HD).transpose(2, 0, 1, 3).astype(BF)
        )
        wk_c = wk[2 * hh * HD:(2 * hh + 2) * HD]           # [256, D]
        wk_p = np.ascontiguousarray(wk_c.T.reshape(c.DT, 128, 2 * HD).astype(BF))
        wv_c = wv[2 * hh * HD:(2 * hh + 2) * HD]
        wv_p = np.ascontiguousarray(wv_c.T.reshape(c.DT, 128, 2 * HD).astype(BF))
        wo_c = wo[:, hq0 * HD:(hq0 + HQC) * HD]            # [D, 1024]
        wo_p = np.ascontiguousarray(
            wo_c.T.reshape(HQC, 128, c.DT, 128).transpose(2, 1, 0, 3)
            .reshape(c.DT, 128, HQC * HD).astype(BF)
        )
        in_maps.append({
            "xt": xt,
            "wq": wq_p,
            "wk": wk_p,
            "wv": wv_p,
            "wo": wo_p,
            "ra": ra,
            "rb": rb,
            "cm": cm,
            "pm": pm,
            "idn": np.eye(128, dtype=BF),
            "onec": np.ones((HD, 1), BF),
            "oner": np.ones((1, HD), BF),
        })
    return in_maps


def run(inputs: dict, n_cores: int = 8, cfg: Cfg = Cfg(), trace: bool = False):
    in_maps = host_prep(
        inputs["x"], inputs["freq_cis"], inputs["wq"], inputs["wk"],
        inputs["wv"], inputs["wo"], n_cores, cfg,
    )
    nc = build_program(cfg)
    res = run_bass_kernel_spmd(nc, in_maps, list(range(n_cores)), trace=trace)
    out = np.empty((cfg.B, cfg.S, cfg.D), np.float32)
    for b in range(cfg.B):
        pa = np.asarray(res.results[2 * b]["partialT"]).astype(np.float32)
        pb = np.asarray(res.results[2 * b + 1]["partialT"]).astype(np.float32)
        out[b] = (pa + pb).T
    return out, res


def kernel(**inputs) -> np.ndarray:
    out, _ = run(inputs, n_cores=8, cfg=Cfg())
    return out
